# revision 45
# baseline (speedup 1.0000x reference)
"""Multi-Head Latent Attention (MLA) Trainium2 kernel, 8-core head-sharded,
with sequence-sharded latent projections + on-device AllGather.

v2 layout strategy: "transposed world" (contraction on partitions) as v1.
Each core computes the latent down-projections (c_Q, c_KV, k_R) only for its
S/8 = 256-position slice, then three AllGathers (ckv, kr, cq) distribute the
full-length latents to every core.  This de-replicates ~123us of per-core PE
work down to ~15us + gather.  Heads stay sharded 2-per-core; each core emits
a partial out.T summed on the host.

Attention softmax denominators: exp chunks are accumulated on the Vector
engine ([128,QB] adds), partition-summed + broadcast via gpsimd
partition_all_reduce, inverted with reciprocal_approx_fast — no PE sum/bcast
matmuls at all.
"""
import sys

sys.path.insert(0, "/opt/trn_rl_repo")

import numpy as np

import concourse.bass as bass
import concourse.tile as tile
from concourse import bacc, bass_isa, mybir
from concourse.bass_utils import run_bass_kernel_spmd

F32 = mybir.dt.float32
F32R = mybir.dt.float32r
BF16 = mybir.dt.bfloat16
AF = mybir.ActivationFunctionType
OP = mybir.AluOpType
RED = bass_isa.ReduceOp

N_CORES = 8
S = 2048          # sequence length
DM = 2048         # d_model
DL = 512          # d_latent
H = 16            # total heads
HC = H // N_CORES  # heads per core (2)
DH = 128          # head dim (content)
DHR = 64          # head dim (rope)
QB = 512          # query block
NQB = S // QB     # 4
KPB = QB // 128   # key chunks per query block (4)
SL = S // N_CORES  # per-core stage1 slice (256)
NMC = DM // 128   # 16 model chunks
NLC = DL // 128   # 4 latent chunks
NKC = S // 128    # 16 key chunks
THETA = 10000.0

SCALE = float(1.0 / np.sqrt(np.float32(DH + DHR)))
E_HI = float(np.exp(np.float64(80.0) * SCALE))
E_LO = float(np.exp(np.float64(-80.0) * SCALE))

# Set by test.py to profile; harness path leaves these untouched.
TRACE = False
TRACE_KWARGS = {}
LAST_EXEC_TIME_NS = None
LAST_RESULTS = None

_CACHE = {}
MM_LABELS = {}

RG = [list(range(N_CORES))]


def _lbl(inst, label):
    try:
        MM_LABELS[inst.ins.name] = label
    except Exception:
        try:
            MM_LABELS[inst.name] = label
        except Exception:
            pass
    return inst


def _build():
    nc = bacc.Bacc("TRN2", target_bir_lowering=False, debug=False,
                   enable_asserts=True, num_devices=N_CORES)

    def din(name, shape, dt=F32R):
        return nc.dram_tensor(name, shape, dt, kind="ExternalInput").ap()

    def dint(name, shape, dt=F32R, shared=False):
        return nc.dram_tensor(
            name, shape, dt, kind="Internal",
            addr_space="Shared" if shared else "Local").ap()

    d = {
        "xsl": din("xsl", [DM, SL], BF16),
        "wdqT": din("wdqT", [DM, DL], BF16),
        "wdkvT": din("wdkvT", [DM, DL], BF16),
        "wkrT": din("wkrT", [DM, DHR], BF16),
        "wuqT": din("wuqT", [DL, HC * DH], BF16),
        "wqrT": din("wqrT", [DL, HC * DHR], BF16),
        "wukT": din("wukT", [DL, HC * DH], BF16),
        "wuvT": din("wuvT", [DL, HC * DH], BF16),
        "woT": din("woT", [HC * DH, DM]),
        "masktri": din("masktri", [128, 128], BF16),
        "ident": din("ident", [128, 128], BF16),
        "permsw": din("permsw", [128, 128]),
        "sel64": din("sel64", [128, DHR], BF16),
        "ones128": din("ones128", [128, 1]),
        "ones1": din("ones1", [1, 128]),
        "cs1d": din("cs1d", [128, S], F32),
        "cs2d": din("cs2d", [128, S], F32),
        "cskr1": din("cskr1", [DHR, SL], F32),
        "cskr2": din("cskr2", [DHR, SL], F32),
        # collective buffers
        "warmL": dint("warmL", [1, 16], F32),
        "warmG": dint("warmG", [N_CORES, 16], F32, shared=True),
        "ckvL": dint("ckvL", [DL, SL], BF16),
        "cqL": dint("cqL", [DL, SL], BF16),
        "krL": dint("krL", [DHR, SL], BF16),
        "ckvG": dint("ckvG", [N_CORES * DL, SL], BF16, shared=True),
        "cqG": dint("cqG", [N_CORES * DL, SL], BF16, shared=True),
        "krG": dint("krG", [N_CORES * DHR, SL], BF16, shared=True),
        "outT": nc.dram_tensor("outT", [DM, S], BF16,
                               kind="ExternalOutput").ap(),
    }
    with tile.TileContext(nc) as tc:
        import contextlib
        with contextlib.ExitStack() as ctx:
            _kernel_body(ctx, tc, nc, d)
    nc.compile()
    return nc


def _kernel_body(ctx, tc, nc, d):
    wts = ctx.enter_context(tc.tile_pool(name="wts", bufs=1))
    kvp = ctx.enter_context(tc.tile_pool(name="kvp", bufs=1))
    xtp = ctx.enter_context(tc.tile_pool(name="xtp", bufs=1))
    lat = ctx.enter_context(tc.tile_pool(name="lat", bufs=1))
    prj = ctx.enter_context(tc.tile_pool(name="prj", bufs=1))
    smp = ctx.enter_context(tc.tile_pool(name="smp", bufs=1))
    o5p = ctx.enter_context(tc.tile_pool(name="o5p", bufs=5))
    str_p = ctx.enter_context(tc.tile_pool(name="str_p", bufs=1))
    # PSUM: 5 rotating work tags + 2 attention accumulators = 7 banks
    ps_rot = ctx.enter_context(tc.tile_pool(name="ps_rot", bufs=1,
                                            space="PSUM"))
    ps_at = ctx.enter_context(tc.tile_pool(name="ps_at", bufs=2,
                                           space="PSUM"))
    ps_sm = ctx.enter_context(tc.tile_pool(name="ps_sm", bufs=1,
                                           space="PSUM"))

    rot = [0]

    def s1tile(shape, name):
        t = ps_rot.tile(shape, F32, tag=f"rot{rot[0] % 4}", name=name)
        rot[0] += 1
        return t

    # ---- small persistent loads ----
    mask_t = wts.tile([128, 128], BF16, name="masktri")
    nc.sync.dma_start(mask_t[:], d["masktri"][:, :])
    ident_t = wts.tile([128, 128], BF16, name="ident")
    nc.sync.dma_start(ident_t[:], d["ident"][:, :])
    perm_t = wts.tile([128, 128], F32R, name="permsw")
    nc.sync.dma_start(perm_t[:], d["permsw"][:, :])
    sel_t = wts.tile([128, DHR], BF16, name="sel64")
    nc.sync.dma_start(sel_t[:], d["sel64"][:, :])
    o128_t = wts.tile([128, 1], F32R, name="o128")
    o1_t = wts.tile([1, 128], F32R, name="o1")
    nc.sync.dma_start(o128_t[:], d["ones128"][:, :])
    nc.sync.dma_start(o1_t[:], d["ones1"][:, :])
    # tiny warmup AllGather: pays the one-time mesh-init cost during stage1
    nc.gpsimd.dma_start(d["warmL"][:, :], o1_t[:, 0:16])
    nc.gpsimd.collective_compute(
        "AllGather", OP.bypass, replica_groups=RG,
        ins=[d["warmL"][:, :]], outs=[d["warmG"][:, :]])

    # head-sharded weights (gpsimd DMA queue, parallel to stage1 streams)
    wuq_t = [wts.tile([128, HC * DH], BF16, name=f"wuq{l}") for l in range(NLC)]
    wqr_t = [wts.tile([128, HC * DHR], BF16, name=f"wqr{l}") for l in range(NLC)]
    wuk_t = [wts.tile([128, HC * DH], BF16, name=f"wuk{l}") for l in range(NLC)]
    wuv_t = [wts.tile([128, HC * DH], BF16, name=f"wuv{l}") for l in range(NLC)]
    wo_t = [wts.tile([128, DM], F32R, name=f"wo{h}") for h in range(HC)]

    def emit_head_weight_dmas():
        for l in range(NLC):
            sl = slice(l * 128, (l + 1) * 128)
            nc.scalar.dma_start(wuk_t[l][:], d["wukT"][sl, :])
            nc.scalar.dma_start(wuv_t[l][:], d["wuvT"][sl, :])
            nc.scalar.dma_start(wuq_t[l][:], d["wuqT"][sl, :])
            nc.scalar.dma_start(wqr_t[l][:], d["wqrT"][sl, :])
        for h in range(HC):
            nc.scalar.dma_start(wo_t[h][:], d["woT"][h * 128:(h + 1) * 128, :])

    # ---- persistent per-sequence state ----
    kct = [kvp.tile([128, S], F32R, name=f"kct{h}") for h in range(HC)]
    krt = kvp.tile([DHR, S], BF16, name="krt")
    vt = [kvp.tile([128, HC * DH], F32R, name=f"vt{k}") for k in range(NKC)]

    def rope(raw_ps, out_ap, cs1s, cs2s, tag, p, w):
        """raw_ps: PSUM [p, w] pre-rope; out_ap dest [p, w]. The 32<->32
        half-swap runs on the PE (permutation matmul) — no DMA latency."""
        raw = smp.tile([p, w], F32R, tag=f"rr{p}", bufs=1, name=f"rr_{tag}")
        nc.vector.tensor_copy(raw[:], raw_ps[:])
        prs = s1tile([p, w], f"prs_{tag}")
        _lbl(nc.tensor.matmul(prs[:], perm_t[0:p, 0:p], raw[:],
                              start=True, stop=True), "ropesw")
        rsw = smp.tile([p, w], F32, tag=f"rs{p}", bufs=1, name=f"rs_{tag}")
        nc.vector.tensor_copy(rsw[:], prs[:])
        nc.vector.tensor_tensor(raw[:], raw[:], cs1s[:], op=OP.mult)
        nc.vector.tensor_tensor(rsw[:], rsw[:], cs2s[:], op=OP.mult)
        nc.vector.tensor_tensor(out_ap, raw[:], rsw[:], op=OP.add)

    # ---- stage 1: local-slice latents + AllGathers ----
    wdkv_t = [wts.tile([128, DL], BF16, name=f"wdkv{m}") for m in range(NMC)]
    wdq_t = [wts.tile([128, DL], BF16, name=f"wdq{m}") for m in range(NMC)]
    wkr_t = [wts.tile([128, DHR], BF16, name=f"wkr{m}") for m in range(NMC)]

    def stage1_local():
        """Fused stage-1: slice latents (for the gather) + replicated
        block-0 latents, sharing stationary weights. Three m-loops:
          A: slice-ckv(rot0-3) qb0-ckv l01(at) kr-slice(sm0) kr-qb0(sm1)
          B: qb0-ckv l23(rot0-1) slice-cq(rot2-3, at0-1)
          C: qb0-cq(rot0-3)
        """
        xt = [xtp.tile([128, SL], BF16, name=f"xt{m}") for m in range(NMC)]
        xq = [xtp.tile([128, QB], BF16, name=f"xq{m}") for m in range(NMC)]
        cskr1 = smp.tile([DHR, SL], F32, tag="cskr1", name="cskr1")
        cskr2 = smp.tile([DHR, SL], F32, tag="cskr2", name="cskr2")
        nc.scalar.dma_start(cskr1[:], d["cskr1"][:, :])
        nc.scalar.dma_start(cskr2[:], d["cskr2"][:, :])
        csk1 = smp.tile([DHR, QB], F32, tag="csk1b", name="csk1b")
        csk2 = smp.tile([DHR, QB], F32, tag="csk2b", name="csk2b")
        nc.scalar.dma_start(csk1[:], d["cs1d"][0:DHR, 0:QB])
        nc.scalar.dma_start(csk2[:], d["cs2d"][0:DHR, 0:QB])
        ckv0 = [lat.tile([128, QB], BF16, tag=f"ckv{l}", bufs=2,
                         name=f"ckv{l}_b0") for l in range(NLC)]
        cq0 = [lat.tile([128, QB], BF16, tag=f"cq{l}", bufs=2,
                        name=f"cq{l}_b0") for l in range(NLC)]

        # ---- pass A ----
        pckv = [s1tile([128, SL], f"pckv{i}") for i in range(NLC)]
        pckv0a = [ps_at.tile([128, QB], F32, tag="at", name=f"pckv0a{i}")
                  for i in range(2)]
        pkr = ps_sm.tile([DHR, SL], F32, tag="sm0", name="pkr")
        pkr0 = ps_sm.tile([DHR, QB], F32, tag="sm1", name="pkr0")
        for m in range(NMC):
            msl = slice(m * 128, (m + 1) * 128)
            nc.scalar.dma_start(xt[m][:], d["xsl"][msl, :])
            nc.sync.dma_start(xq[m][:], d["xq0"][msl, :])
            nc.sync.dma_start(wdkv_t[m][:], d["wdkvT"][msl, :])
            nc.gpsimd.dma_start(wkr_t[m][:], d["wkrT"][msl, :])
            st, sp = (m == 0), (m == NMC - 1)
            for l in range(NLC):
                _lbl(nc.tensor.matmul(
                    pckv[l][:], wdkv_t[m][:, l * 128:(l + 1) * 128], xt[m][:],
                    start=st, stop=sp), "s1_ckv")
                if l < 2:  # same stationary, qb0 moving
                    _lbl(nc.tensor.matmul(
                        pckv0a[l][:], wdkv_t[m][:, l * 128:(l + 1) * 128],
                        xq[m][:], start=st, stop=sp), "s1_ckv")
            _lbl(nc.tensor.matmul(pkr[:], wkr_t[m][:], xt[m][:],
                                  start=st, stop=sp), "s1_kr")
            _lbl(nc.tensor.matmul(pkr0[:], wkr_t[m][:], xq[m][:],
                                  start=st, stop=sp), "s1_kr")
        for l in range(NLC):
            ck = smp.tile([128, SL], BF16, tag="ckL", bufs=2, name=f"ckL{l}")
            (nc.vector.tensor_copy if l % 2 == 0 else nc.scalar.copy)(
                ck[:], pckv[l][:])
            nc.gpsimd.dma_start(d["latL"][l * 128:(l + 1) * 128, :], ck[:])
        for i in range(2):
            (nc.vector.tensor_copy if i == 0 else nc.scalar.copy)(
                ckv0[i][:], pckv0a[i][:])
        krl = smp.tile([DHR, SL], BF16, tag="krl", name="krl")
        rope(pkr, krl[:], cskr1, cskr2, "krloc", DHR, SL)
        nc.gpsimd.dma_start(d["latL"][DL:DL + DHR, :], krl[:])
        rope(pkr0, krt[:, 0:QB], csk1, csk2, "kr_b0", DHR, QB)

        # ---- pass B ----
        pckv0b = [s1tile([128, QB], f"pckv0b{i}") for i in range(2)]
        pcq = [s1tile([128, SL], f"pcq{i}") for i in range(2)]
        pcqa = [ps_at.tile([128, QB], F32, tag="at", name=f"pcqa{i}")
                for i in range(2)]
        for m in range(NMC):
            msl = slice(m * 128, (m + 1) * 128)
            nc.sync.dma_start(wdq_t[m][:], d["wdqT"][msl, :])
            st, sp = (m == 0), (m == NMC - 1)
            for i in range(2):
                _lbl(nc.tensor.matmul(
                    pckv0b[i][:], wdkv_t[m][:, (2 + i) * 128:(3 + i) * 128],
                    xq[m][:], start=st, stop=sp), "s1_ckv")
            for l in range(NLC):
                tgt = pcq[l] if l < 2 else pcqa[l - 2]
                _lbl(nc.tensor.matmul(
                    tgt[:, 0:SL] if l >= 2 else tgt[:],
                    wdq_t[m][:, l * 128:(l + 1) * 128], xt[m][:],
                    start=st, stop=sp), "s1_cq")
        for i in range(2):
            (nc.vector.tensor_copy if i == 0 else nc.scalar.copy)(
                ckv0[2 + i][:], pckv0b[i][:])
        for l in range(NLC):
            cqs = smp.tile([128, SL], BF16, tag="cqLs", bufs=2,
                           name=f"cqL{l}")
            srcp = pcq[l][:] if l < 2 else pcqa[l - 2][:, 0:SL]
            (nc.vector.tensor_copy if l % 2 == 0 else nc.scalar.copy)(
                cqs[:], srcp)
            nc.gpsimd.dma_start(
                d["latL"][DL + DHR + l * 128:DL + DHR + (l + 1) * 128, :],
                cqs[:])
        nc.gpsimd.collective_compute(
            "AllGather", OP.bypass, replica_groups=RG,
            ins=[d["latL"][:, :]], outs=[d["latG"][:, :]])

        # ---- pass C ----
        pcq0 = [s1tile([128, QB], f"pcq0{i}") for i in range(NLC)]
        for m in range(NMC):
            st, sp = (m == 0), (m == NMC - 1)
            for l in range(NLC):
                _lbl(nc.tensor.matmul(
                    pcq0[l][:], wdq_t[m][:, l * 128:(l + 1) * 128], xq[m][:],
                    start=st, stop=sp), "s1_cq")
        for l in range(NLC):
            (nc.vector.tensor_copy if l % 2 == 0 else nc.scalar.copy)(
                cq0[l][:], pcq0[l][:])
        return ckv0, cq0

    def load_krt():
        for b in range(N_CORES):
            nc.gpsimd.dma_start(krt[:, b * SL:(b + 1) * SL],
                              d["krG"][b * DHR:(b + 1) * DHR, :])

    # ---- stage 2: per-block projections from gathered latents ----
    def stage2kv(qb):
        """k_C / V for block qb — depends only on ckvG (+wuk/wuv)."""
        qsl = slice(qb * QB, (qb + 1) * QB)
        b0, b1 = 2 * qb, 2 * qb + 1
        ckv = [lat.tile([128, QB], BF16, tag=f"ckv{l}", bufs=2,
                        name=f"ckv{l}_{qb}") for l in range(NLC)]
        for l in range(NLC):
            nc.sync.dma_start(
                ckv[l][:, 0:SL],
                d["ckvG"][b0 * DL + l * 128:b0 * DL + (l + 1) * 128, :])
            nc.sync.dma_start(
                ckv[l][:, SL:QB],
                d["ckvG"][b1 * DL + l * 128:b1 * DL + (l + 1) * 128, :])
        for h in range(HC):
            pkc = s1tile([128, QB], f"pkc{h}_{qb}")
            for l in range(NLC):
                _lbl(nc.tensor.matmul(pkc[:], wuk_t[l][:, h * DH:(h + 1) * DH],
                                      ckv[l][:], start=(l == 0),
                                      stop=(l == NLC - 1)), "s2_kc")
            (nc.vector.tensor_copy if h == 0 else nc.scalar.copy)(
                kct[h][:, qsl], pkc[:])
        for sc in range(KPB):
            k = qb * KPB + sc
            pvv = s1tile([128, HC * DH], f"pvv{k}")
            for l in range(NLC):
                _lbl(nc.tensor.matmul(pvv[:],
                                      ckv[l][:, sc * 128:(sc + 1) * 128],
                                      wuv_t[l][:], start=(l == 0),
                                      stop=(l == NLC - 1)), "s2_v")
            (nc.vector.tensor_copy if sc % 2 == 0 else nc.scalar.copy)(
                vt[k][:], pvv[:])

    def stage2q(qb):
        """q_C / q_R for block qb — depends on cqG (+wuq/wqr)."""
        qsl = slice(qb * QB, (qb + 1) * QB)
        b0, b1 = 2 * qb, 2 * qb + 1
        cq = [lat.tile([128, QB], BF16, tag=f"cq{l}", bufs=2,
                       name=f"cq{l}_{qb}") for l in range(NLC)]
        for l in range(NLC):
            nc.sync.dma_start(
                cq[l][:, 0:SL],
                d["cqG"][b0 * DL + l * 128:b0 * DL + (l + 1) * 128, :])
            nc.sync.dma_start(
                cq[l][:, SL:QB],
                d["cqG"][b1 * DL + l * 128:b1 * DL + (l + 1) * 128, :])
        qct = [prj.tile([128, QB], F32R, tag=f"qct{h}", bufs=2,
                        name=f"qct{h}_{qb}") for h in range(HC)]
        for h in range(HC):
            pqc = s1tile([128, QB], f"pqc{h}_{qb}")
            for l in range(NLC):
                _lbl(nc.tensor.matmul(pqc[:], wuq_t[l][:, h * DH:(h + 1) * DH],
                                      cq[l][:], start=(l == 0),
                                      stop=(l == NLC - 1)), "s2_qc")
            (nc.vector.tensor_copy if h == 0 else nc.scalar.copy)(
                qct[h][:], pqc[:])
        cs1s = smp.tile([128, QB], F32, tag="cs1s", bufs=1, name=f"cs1s{qb}")
        cs2s = smp.tile([128, QB], F32, tag="cs2s", bufs=1, name=f"cs2s{qb}")
        nc.sync.dma_start(cs1s[:], d["cs1d"][:, qsl])
        nc.sync.dma_start(cs2s[:], d["cs2d"][:, qsl])
        qrt = prj.tile([128, QB], BF16, tag="qrt", bufs=2, name=f"qrt_{qb}")
        pqr = s1tile([128, QB], f"pqr_{qb}")
        for l in range(NLC):
            _lbl(nc.tensor.matmul(pqr[:], wqr_t[l][:, 0:HC * DHR], cq[l][:],
                                  start=(l == 0), stop=(l == NLC - 1)),
                 "s2_qr")
        rope(pqr, qrt[:], cs1s, cs2s, f"qr{qb}", 128, QB)
        # matmul moving operands must be base-partition-0: split head 1 out
        qrt1 = prj.tile([DHR, QB], BF16, tag="qrt1", bufs=2,
                        name=f"qrt1_{qb}")
        nc.scalar.dma_start(qrt1[:], qrt[DHR:2 * DHR, :])
        return qct, (qrt, qrt1)

    # ---- attention ----
    def attn_both(qb, qct, qrt_pair, s5fill=None):
        nkc = KPB * (qb + 1)
        pat = [ps_at.tile([128, QB], F32, tag="at", name=f"pat{h}_{qb}")
               for h in range(HC)]
        psums = [ps_sm.tile([1, QB], F32, tag=f"sm{h}", name=f"psums{h}_{qb}")
                 for h in range(HC)]
        pend = []  # (h, kc, off, pt) awaiting sum+PV

        def flush(last):
            h, kc, off, pt = pend.pop(0)
            _lbl(nc.tensor.matmul(psums[h][:, off:], o128_t[:], pt[:, off:],
                                  start=(kc == 0), stop=last,
                                  skip_group_check=True), "sum")
            _lbl(nc.tensor.matmul(pat[h][:, off:],
                                  vt[kc][:, h * DH:(h + 1) * DH],
                                  pt[:, off:], start=(kc == 0), stop=last,
                                  skip_group_check=True), "pv")

        for kc in range(nkc):
            off = 128 * (kc - KPB * qb) if kc >= KPB * qb else 0
            w = QB - off
            ksl = slice(kc * 128, (kc + 1) * 128)
            for h in range(HC):
                diag = kc >= KPB * qb
                ps_s = s1tile([128, QB], f"s{h}_{qb}_{kc}")
                _lbl(nc.tensor.matmul(ps_s[:, off:], kct[h][:, ksl],
                                      qct[h][:, off:], start=True, stop=False,
                                      skip_group_check=True), "qk_c")
                qr_mv = (qrt_pair[0][0:DHR, off:] if h == 0
                         else qrt_pair[1][:, off:])
                _lbl(nc.tensor.matmul(ps_s[:, off:], krt[:, ksl], qr_mv,
                                      start=False, stop=(not diag),
                                      skip_group_check=True), "qk_r")
                if diag:  # causal bias for the 128-wide diagonal window
                    _lbl(nc.tensor.matmul(ps_s[:, off:off + 128], ident_t[:],
                                          mask_t[:], start=False, stop=True,
                                          skip_group_check=True), "maskb")
                if len(pend) >= 2:
                    flush(False)
                if s5fill is not None:
                    s5fill()
                et = smp.tile([128, QB], F32, tag="et", bufs=3,
                              name=f"et{h}_{qb}_{kc}")
                nc.scalar.activation(et[:, off:], ps_s[:, off:], AF.Exp,
                                     scale=SCALE)
                pt = smp.tile([128, QB], F32R, tag="pt", bufs=4,
                              name=f"pt{h}_{qb}_{kc}")
                ceng = nc.vector if h == 0 else nc.gpsimd
                ceng.tensor_scalar(pt[:, off:], et[:, off:], E_HI, 0.0,
                                   op0=OP.min, op1=OP.max)
                pend.append((h, kc, off, pt))
        while len(pend) > 2:
            flush(False)
        while pend:
            flush(True)
        return pat, psums

    def attn_sum(qb, h, psum):
        """Approx reciprocal of the PSUM-accumulated denominators."""
        rc = smp.tile([1, QB], F32, tag=f"rc{h}", bufs=1, name=f"rc{h}_{qb}")
        nc.vector.reciprocal_approx_fast(rc[:], psum[:])
        rcr = smp.tile([1, QB], F32R, tag=f"rcr{h}", bufs=1,
                       name=f"rcr{h}_{qb}")
        nc.scalar.copy(rcr[:], rc[:])
        return rcr

    def attn_bcast(qb, h, rcr):
        prb = s1tile([128, QB], f"prb{h}_{qb}")
        _lbl(nc.tensor.matmul(prb[:], o1_t[:], rcr[:], start=True,
                              stop=True), "bcast")
        rbs = smp.tile([128, QB], F32R, tag=f"rbs{h}", bufs=1,
                       name=f"rbs{h}_{qb}")
        (nc.scalar.copy if h == 0 else nc.vector.tensor_copy)(rbs[:], prb[:])
        return rbs

    def make_s5_filler(qb, attn_n):
        """Returns (fill, drain): fill() emits one out-projection chunk,
        called from inside the next block's attention loop so the PSUM->SBUF
        copies hide under qk/exp work."""
        state = [0]
        skip = [5]

        def fill():
            if skip[0] > 0:
                skip[0] -= 1
                return False
            m = state[0]
            if m >= NMC:
                return False
            state[0] += 1
            qsl = slice(qb * QB, (qb + 1) * QB)
            po = s1tile([128, QB], f"po{m}_{qb}")
            for h in range(HC):
                _lbl(nc.tensor.matmul(po[:],
                                      wo_t[h][:, m * 128:(m + 1) * 128],
                                      attn_n[h][:], start=(h == 0),
                                      stop=(h == HC - 1)), "s5")
            ob = o5p.tile([128, QB], BF16, tag="ob", name=f"ob{m}_{qb}")
            (nc.vector.tensor_copy if m % 2 == 0 else nc.scalar.copy)(
                ob[:], po[:])
            nc.sync.dma_start(d["outT"][m * 128:(m + 1) * 128, qsl], ob[:])
            return True

        def drain():
            skip[0] = 0
            while state[0] < NMC:
                fill()

        return fill, drain

    # ---- main flow ----
    stage1_local()
    emit_head_weight_dmas()
    load_krt()
    for qb in range(NQB):
        stage2kv(qb)   # kv cache built as soon as ckvG lands; hides cq gather
    qct, qrt = stage2q(0)
    s5 = None
    for qb in range(NQB):
        pat, psums = attn_both(qb, qct, qrt,
                               s5fill=s5[0] if s5 else None)
        if s5 is not None:
            s5[1]()    # drain any leftover chunks of the previous block
        rcr = [attn_sum(qb, h, psums[h]) for h in range(HC)]
        nxt = stage2q(qb + 1) if qb < NQB - 1 else None
        rbs = [attn_bcast(qb, h, rcr[h]) for h in range(HC)]
        attn_n = [prj.tile([128, QB], F32R, tag=f"an{h}", bufs=2,
                           name=f"an{h}_{qb}") for h in range(HC)]
        for h in range(HC):
            nc.vector.tensor_tensor(attn_n[h][:], pat[h][:], rbs[h][:],
                                    op=OP.mult)
        s5 = make_s5_filler(qb, attn_n)
        if nxt is not None:
            qct, qrt = nxt
    s5[1]()            # final block's out-projection
